# revision 31
# baseline (speedup 1.0000x reference)
"""GeneSAGE (2-layer GraphSAGE + skip + LayerNorm + ELU) on 8 Trainium2 cores.

v2 design. Edge-parallel by destination range: core c owns nodes
[cp*c, cp*(c+1)), cp=6272. Both convs share one edge bucketing: edges
are grouped by (src-half stream, 64-node dst window) with chunks of 128
edge slots, chunk structure common to all 8 cores (SPMD). Per chunk a
[128, 64] one-hot (batched generation: 16 chunks per DVE instruction
via a broadcast is_equal against a resident iota) is matmul'd against
the gathered per-edge payload, accumulating node-major [64dst, *] PSUM
strips which are added into an SBUF agg. Gathers use SWDGE dma_gather
spread round-robin over 4 queues (desc-gen parallelizes across Q7 core
pairs; this is the dominant cost). Conv1 gathers x rows (256B fp32)
from HBM; conv2 gathers 256B rows of a device-built table where row n
= 32 copies of (p0[n], p1[n]), so the matmul rhs is a free 2-column
slice (p = h @ W2l is computed per window in the conv1 dense phase,
AllGather'd, then expanded/written to HBM). The dense phase
(mean/linear/LayerNorm/ELU/p,r) runs per 128-node window interleaved
under the conv1 edge stream with deferred epilogues; LayerNorm mean /
center / variance and PSUM moves run on the Scalar engine.
"""

import numpy as np

import concourse.mybir as mybir
from concourse import bacc, bass, tile
from concourse.bass_utils import run_bass_kernel_spmd

F32 = mybir.dt.float32
I16 = mybir.dt.int16

N_CORES = 8
D = 64
HID = 256
OUT = 2
LN_EPS = 1e-5
B = 32            # chunks per dma_gather
G = 16            # chunks per one-hot DVE instruction
N_SWDGE_Q = 4
DEBUG = False
AF = mybir.ActivationFunctionType
OP = mybir.AluOpType


def make_plan(edge_index: np.ndarray, n_nodes: int):
    cp = int(np.ceil(n_nodes / (N_CORES * 128))) * 128
    nw = cp // 128
    nw64 = cp // 64
    npad = N_CORES * cp
    half = npad // 2
    assert half <= 32768

    src = edge_index[0].astype(np.int64)
    dst = edge_index[1].astype(np.int64)
    E = src.shape[0]

    deg = np.bincount(dst, minlength=npad).astype(np.float64)
    rc = (1.0 / np.maximum(deg, 1.0)).astype(np.float32)
    rc_tile = rc.reshape(N_CORES, nw, 128).transpose(0, 2, 1).copy()

    core = dst // cp
    win64 = (dst % cp) // 64
    parity = (src & 1).astype(np.int64)
    ngrp = 2 * nw64                # window-major groups: (win64, parity)
    g = win64 * 2 + parity
    key = core * ngrp + g
    order = np.argsort(key, kind="stable")
    counts = np.bincount(key, minlength=N_CORES * ngrp).reshape(
        N_CORES, ngrp)
    nch = -(-counts.max(axis=0) // 128)          # [ngrp] common structure
    off = np.zeros(ngrp, np.int64)
    running = 0
    for gg in range(ngrp):
        off[gg] = running
        running += nch[gg]
    c_total = int(running)
    e_slots = c_total * 128

    sk = key[order]
    grp_start = np.searchsorted(sk, np.arange(N_CORES * ngrp))
    rank = np.arange(E) - grp_start[sk]
    g_of = sk % ngrp
    c_of = sk // ngrp
    slot = off[g_of] * 128 + rank

    gidx = np.zeros((N_CORES, e_slots), np.int16)
    dstf = np.full((N_CORES, e_slots), -1.0, np.float32)
    gidx[c_of, slot] = (src[order] >> 1).astype(np.int16)
    dstf[c_of, slot] = (dst[order] % 64).astype(np.float32)

    a = gidx.reshape(N_CORES, e_slots // 16, 16).transpose(0, 2, 1)
    gidx_tile = np.tile(a, (1, 8, 1)).copy()      # [c, 128, J]
    dstf_tile = dstf.reshape(N_CORES, c_total, 128).transpose(0, 2, 1).copy()

    # per-chunk info: (w, s2, par, strip_start, strip_stop). A strip
    # (w, s2) covers both parity groups of win64=2w+s2, which are
    # contiguous in chunk order, so PSUM accumulates per strip directly.
    chunk_info = [None] * c_total
    last_chunk = np.full(nw, -1, np.int64)
    for w in range(nw):
        for s2 in range(2):
            w64 = 2 * w + s2
            present = [w64 * 2 + p for p in range(2) if nch[w64 * 2 + p] > 0]
            if not present:
                continue
            first = int(off[present[0]])
            last = int(off[present[-1]] + nch[present[-1]] - 1)
            for gg in present:
                par = gg % 2
                for c in range(int(off[gg]), int(off[gg] + nch[gg])):
                    chunk_info[c] = (w, s2, par, c == first, c == last)
            last_chunk[w] = max(last_chunk[w], last)
    fire = {}
    for w in range(nw):
        if last_chunk[w] >= 0:
            fire.setdefault(int(last_chunk[w]), []).append(w)

    return dict(cp=cp, nw=nw, nw64=nw64, npad=npad, half=half,
                c_total=c_total, gidx_tile=gidx_tile, dstf_tile=dstf_tile,
                rc_tile=rc_tile, chunk_info=chunk_info, fire=fire)


def build_program(plan):
    cp, nw, npad, half = plan["cp"], plan["nw"], plan["npad"], plan["half"]
    c_total = plan["c_total"]
    J = c_total * 8
    npair = npad // 2
    nblk = npair // 128         # 196 pair-rows per partition
    chunk_info = plan["chunk_info"]
    fire = plan["fire"]

    nc = bacc.Bacc("TRN2", target_bir_lowering=False, debug=False,
                   num_devices=N_CORES, num_swdge_queues=N_SWDGE_Q)

    def inp(name, shape, dt=F32):
        return nc.dram_tensor(name, shape, dt, kind="ExternalInput").ap()

    x2_d = inp("x2", [npair, 2 * D])
    xt_d = inp("xt", [D + 1, cp])
    gidx_d = inp("gidx", [128, J], I16)
    dstf_d = inp("dstf", [128, c_total])
    iota_d = inp("iota64", [128, G, 64])
    ident_d = inp("ident", [128, 128])
    wcb_d = inp("wcb", [D + 1, HID])
    w1l_d = inp("w1l", [D, HID])
    w2lr_d = inp("w2lr", [128, 2 * 2 * OUT])
    gamma_d = inp("gamma_bc", [128, HID])
    beta_d = inp("beta_bc", [128, HID])
    b2_d = inp("b2_bc", [128, OUT])
    rc_d = inp("rc", [128, nw])
    out_d = nc.dram_tensor("out", [cp, OUT], F32, kind="ExternalOutput").ap()
    if DEBUG:
        dagg_d = nc.dram_tensor("dagg", [cp, D], F32,
                                kind="ExternalOutput").ap()
        dpr_d = nc.dram_tensor("dpr", [cp, 2 * OUT], F32,
                               kind="ExternalOutput").ap()
        dpa_d = nc.dram_tensor("dpa", [128, nblk * 2 * OUT], F32,
                               kind="ExternalOutput").ap()

    with tile.TileContext(nc) as tc:
        with (
            tc.tile_pool(name="res", bufs=1) as res,
            tc.tile_pool(name="dram", bufs=1, space="DRAM") as dram,
        ):
            def load(name, shape, src, dt=F32):
                t = res.tile(shape, dt, tag=name)
                nc.sync.dma_start(out=t[:], in_=src[:])
                return t

            gidx_sb = load("gidx", [128, J], gidx_d, I16)
            dstf_sb = load("dstf", [128, c_total], dstf_d)
            iota_sb = load("iota", [128, G, 64], iota_d)
            ident_sb = load("ident", [128, 128], ident_d)
            xt_sb = load("xt", [D + 1, cp], xt_d)
            wcb_sb = load("wcb", [D + 1, HID], wcb_d)
            w1l_sb = load("w1l", [D, HID], w1l_d)
            w2lr_sb = load("w2lr", [128, 2 * 2 * OUT], w2lr_d)
            gamma_sb = load("gamma", [128, HID], gamma_d)
            beta_sb = load("beta", [128, HID], beta_d)
            b2_sb = load("b2", [128, OUT], b2_d)
            rc_sb = load("rc", [128, nw], rc_d)

            pr_sb = res.tile([128, nw, 2 * OUT], F32, tag="prs")
            out_sb = res.tile([128, nw, OUT], F32, tag="outs")
            pa_sb = res.tile([128, nblk, 2 * OUT], F32, tag="pas")

            pk_local = dram.tile([cp, OUT], F32)
            pk_all = dram.tile([128, nblk, 2 * OUT], F32)
            tab2 = dram.tile([128, nblk, D], F32)
            tab2_flat = tab2[:].rearrange("q s d -> (q s) d")

            qi = [0]

            def emit_gathers(table, rowd, gpool, tag):
                gmap = {}
                for b0 in range(0, c_total, B):
                    b1 = min(b0 + B, c_total)
                    g_t = gpool.tile([128, B, rowd], F32, tag=tag)
                    n_idx = (b1 - b0) * 128
                    nc.gpsimd.dma_gather(
                        out_ap=g_t[:, 0:b1 - b0, :],
                        in_ap=table,
                        idxs_ap=gidx_sb[:, b0 * 8:b1 * 8],
                        num_idxs=n_idx,
                        num_idxs_reg=n_idx,
                        elem_size=rowd,
                        single_packet=False,
                        queue_num=qi[0] % N_SWDGE_Q,
                    )
                    qi[0] += 1
                    for c in range(b0, b1):
                        gmap[c] = (g_t, c - b0)
                return gmap

            def onehot_for(c, opool, obufs):
                cb = (c // G) * G
                if cb not in obufs:
                    o_t = opool.tile([128, G, 64], F32, tag="o")
                    n = min(G, c_total - cb)
                    src3 = dstf_sb[:, cb:cb + n].unsqueeze(2).broadcast_to(
                        [128, n, 64])
                    nc.vector.tensor_tensor(
                        out=o_t[:, 0:n, :], in0=iota_sb[:, 0:n, :],
                        in1=src3, op=OP.is_equal)
                    obufs.clear()
                    obufs[cb] = o_t
                return obufs[cb]

            # =================== conv1 ===================
            with (
                tc.tile_pool(name="gpool", bufs=5) as gpool,
                tc.tile_pool(name="opool", bufs=6) as opool,
                tc.tile_pool(name="pwp", bufs=4, space="PSUM") as pwp,
                tc.tile_pool(name="dps1", bufs=1, space="PSUM") as dps1,
                tc.tile_pool(name="dps2", bufs=1, space="PSUM") as dps2,
                tc.tile_pool(name="dwork", bufs=2) as dwork,
                tc.tile_pool(name="mwork", bufs=8) as mwork,
                tc.tile_pool(name="dsmall", bufs=4) as dsmall,
            ):
                gmap = emit_gathers(x2_d, 2 * D, gpool, "g1")
                cur_mean = {}

                def dense(w):
                    mean_sb = cur_mean.pop(w)
                    tp = dps1.tile([64, 128], F32, tag="tp")
                    nc.tensor.transpose(tp[:], mean_sb[:], ident_sb[:])
                    meant = mwork.tile([64, 128], F32, tag="meant")
                    nc.scalar.activation(meant[:], tp[:], AF.Copy)

                    x1p = dps2.tile([128, HID], F32, tag="x1")
                    nc.tensor.matmul(
                        x1p[:], xt_sb[:, 128 * w:128 * (w + 1)], wcb_sb[:],
                        start=True, stop=False)
                    nc.tensor.matmul(x1p[:], meant[:], w1l_sb[:],
                                     start=False, stop=True)

                    # LayerNorm
                    musum = dsmall.tile([128, 1], F32, tag="mu")
                    nc.vector.reduce_sum(out=musum[:], in_=x1p[:],
                                         axis=mybir.AxisListType.X)
                    mu = dsmall.tile([128, 1], F32, tag="muv")
                    nc.vector.tensor_scalar(
                        out=mu[:], in0=musum[:], scalar1=1.0 / HID,
                        scalar2=None, op0=OP.mult)
                    xc = dwork.tile([128, HID], F32, tag="xc")
                    nc.vector.tensor_scalar(
                        out=xc[:], in0=x1p[:], scalar1=mu[:], scalar2=None,
                        op0=OP.subtract)
                    sq = dwork.tile([128, HID], F32, tag="sq")
                    var = dsmall.tile([128, 1], F32, tag="var")
                    nc.vector.scalar_tensor_tensor(
                        out=sq[:], in0=xc[:], scalar=1.0, in1=xc[:],
                        op0=OP.mult, op1=OP.mult, accum_out=var[:])
                    rstd = dsmall.tile([128, 1], F32, tag="rstd")
                    nc.vector.tensor_scalar(
                        out=rstd[:], in0=var[:], scalar1=1.0 / HID,
                        scalar2=LN_EPS, op0=OP.mult, op1=OP.add)
                    nc.scalar.activation(rstd[:], rstd[:], AF.Sqrt)
                    nc.vector.reciprocal(rstd[:], rstd[:])
                    y = dwork.tile([128, HID], F32, tag="y")
                    nc.vector.scalar_tensor_tensor(
                        out=y[:], in0=xc[:], scalar=rstd[:], in1=gamma_sb[:],
                        op0=OP.mult, op1=OP.mult)
                    nc.vector.tensor_tensor(
                        out=y[:], in0=y[:], in1=beta_sb[:], op=OP.add)
                    # ELU = (max(y,0)-1) + exp(min(y,0))
                    m0 = dwork.tile([128, HID], F32, tag="m0")
                    nc.vector.tensor_scalar(
                        out=m0[:], in0=y[:], scalar1=0.0, scalar2=None,
                        op0=OP.min)
                    ex = dwork.tile([128, HID], F32, tag="ex")
                    nc.scalar.activation(ex[:], m0[:], AF.Exp)
                    rm1 = dwork.tile([128, HID], F32, tag="rm1")
                    nc.vector.tensor_scalar(
                        out=rm1[:], in0=y[:], scalar1=0.0, scalar2=-1.0,
                        op0=OP.max, op1=OP.add)
                    h = dwork.tile([128, HID], F32, tag="h")
                    nc.vector.tensor_tensor(
                        out=h[:], in0=rm1[:], in1=ex[:], op=OP.add)

                    # p | r = h @ [W2l | W2r]
                    prp = dps2.tile([128, 2 * OUT], F32, tag="pr")
                    for hh in range(2):
                        tph = dps1.tile([128, 128], F32, tag="tph")
                        nc.tensor.transpose(
                            tph[:], h[:, 128 * hh:128 * (hh + 1)],
                            ident_sb[:])
                        hts = mwork.tile([128, 128], F32, tag="hts")
                        nc.scalar.activation(hts[:], tph[:], AF.Copy)
                        nc.tensor.matmul(
                            prp[:], hts[:], w2lr_sb[:, 4 * hh:4 * (hh + 1)],
                            start=(hh == 0), stop=(hh == 1))
                    nc.scalar.activation(pr_sb[:, w, :], prp[:], AF.Copy)
                    nc.sync.dma_start(
                        out=pk_local[128 * w:128 * (w + 1), :],
                        in_=pr_sb[:, w, 0:OUT])
                    if DEBUG:
                        nc.sync.dma_start(
                            out=dagg_d[128 * w:128 * (w + 1), :],
                            in_=mean_sb[:])
                        nc.sync.dma_start(
                            out=dpr_d[128 * w:128 * (w + 1), :],
                            in_=pr_sb[:, w, :])

                obufs = {}
                pend_dense = []
                pw = None
                for c in range(c_total):
                    info = chunk_info[c]
                    if info is None:
                        continue
                    w, s2, par, s_start, s_stop = info
                    o_t = onehot_for(c, opool, obufs)
                    g_t, col = gmap[c]
                    if s_start:
                        pw = pwp.tile([128, D], F32, tag="pw")
                    nc.tensor.matmul(
                        pw[64 * s2:64 * s2 + 64, :],
                        o_t[:, c - (c // G) * G, :],
                        g_t[:, col, 64 * par:64 * par + 64],
                        start=s_start, stop=s_stop)
                    if s_stop:
                        if w not in cur_mean:
                            mtile = mwork.tile([128, D], F32, tag="mean")
                            cur_mean[w] = mtile
                        nc.vector.tensor_scalar(
                            out=cur_mean[w][64 * s2:64 * s2 + 64, :],
                            in0=pw[64 * s2:64 * s2 + 64, :],
                            scalar1=rc_sb[64 * s2:64 * s2 + 64, w:w + 1],
                            scalar2=None, op0=OP.mult)
                    if c in fire:
                        for wf in fire[c]:
                            pend_dense.append(wf)
                            if len(pend_dense) == 5:
                                dense(pend_dense.pop(0))
                for wf in pend_dense:
                    dense(wf)

            # =============== AllGather p + build conv2 table ===============
            nc.gpsimd.collective_compute(
                "AllGather",
                OP.bypass,
                replica_groups=[list(range(N_CORES))],
                ins=[pk_local.opt()],
                outs=[pk_all.opt()],
            )
            nc.sync.dma_start(out=pa_sb[:], in_=pk_all[:])
            if DEBUG:
                nc.sync.dma_start(out=dpa_d[:], in_=pa_sb[:])
            with tc.tile_pool(name="tabp", bufs=2) as tabp:
                npiece = 4
                pb = nblk // npiece
                for pc in range(npiece):
                    tp_ = tabp.tile([128, pb, 16, 2 * OUT], F32, tag="tp")
                    srcv = pa_sb[:, pb * pc:pb * (pc + 1), :].unsqueeze(
                        2).broadcast_to([128, pb, 16, 2 * OUT])
                    nc.vector.tensor_copy(out=tp_[:], in_=srcv)
                    nc.sync.dma_start(
                        out=tab2[:, pb * pc:pb * (pc + 1), :], in_=tp_[:])

            # =================== conv2 ===================
            with (
                tc.tile_pool(name="gpool2", bufs=8) as gpool2,
                tc.tile_pool(name="opool2", bufs=6) as opool2,
                tc.tile_pool(name="pw2p", bufs=4, space="PSUM") as pw2p,
                tc.tile_pool(name="ewk", bufs=8) as ewk,
            ):
                gmap2 = emit_gathers(tab2_flat, D, gpool2, "g2")
                cur_t = {}

                def epi2(w):
                    t = cur_t.pop(w)
                    nc.vector.tensor_tensor(
                        out=t[:], in0=t[:], in1=pr_sb[:, w, OUT:2 * OUT],
                        op=OP.add)
                    nc.vector.tensor_tensor(
                        out=out_sb[:, w, :], in0=t[:], in1=b2_sb[:],
                        op=OP.add)
                    nc.sync.dma_start(
                        out=out_d[128 * w:128 * (w + 1), :],
                        in_=out_sb[:, w, :])

                obufs2 = {}
                pend2 = []
                pw2 = None
                for c in range(c_total):
                    info = chunk_info[c]
                    if info is None:
                        continue
                    w, s2, par, s_start, s_stop = info
                    o_t = onehot_for(c, opool2, obufs2)
                    g_t, col = gmap2[c]
                    if s_start:
                        pw2 = pw2p.tile([128, OUT], F32, tag="pw2")
                    nc.tensor.matmul(
                        pw2[64 * s2:64 * s2 + 64, :],
                        o_t[:, c - (c // G) * G, :],
                        g_t[:, col, 2 * par:2 * par + 2],
                        start=s_start, stop=s_stop)
                    if s_stop:
                        if w not in cur_t:
                            ttile = ewk.tile([128, OUT], F32, tag="t")
                            cur_t[w] = ttile
                        nc.vector.tensor_scalar(
                            out=cur_t[w][64 * s2:64 * s2 + 64, :],
                            in0=pw2[64 * s2:64 * s2 + 64, :],
                            scalar1=rc_sb[64 * s2:64 * s2 + 64, w:w + 1],
                            scalar2=None, op0=OP.mult)
                    if c in fire:
                        for wf in fire[c]:
                            pend2.append(wf)
                            if len(pend2) == 5:
                                epi2(pend2.pop(0))
                for wf in pend2:
                    epi2(wf)

    nc.compile()
    return nc


def make_inputs(plan, x, W1l, W1r, b1, Wskip, bskip, gamma, beta, W2l, W2r,
                b2, n_nodes):
    cp, half, npad, nw = plan["cp"], plan["half"], plan["npad"], plan["nw"]
    xp = np.zeros((npad, D), np.float32)
    xp[:n_nodes] = np.asarray(x, np.float32)
    wc = np.asarray(W1r, np.float32) + np.asarray(Wskip, np.float32)
    bc = np.asarray(b1, np.float32) + np.asarray(bskip, np.float32)
    wcb = np.concatenate([wc, bc[None, :]], axis=0)
    w2lr_full = np.concatenate(
        [np.asarray(W2l, np.float32), np.asarray(W2r, np.float32)], axis=1)
    w2lr = (
        w2lr_full.reshape(2, 128, 2 * OUT).transpose(1, 0, 2)
        .reshape(128, 2 * 2 * OUT).copy()
    )
    iota64 = np.broadcast_to(
        np.arange(64, dtype=np.float32)[None, None, :],
        (128, G, 64)).copy()
    ident = np.eye(128, dtype=np.float32)
    gamma_bc = np.tile(np.asarray(gamma, np.float32)[None, :], (128, 1))
    beta_bc = np.tile(np.asarray(beta, np.float32)[None, :], (128, 1))
    b2_bc = np.tile(np.asarray(b2, np.float32)[None, :], (128, 1))

    common = dict(
        x2=xp.reshape(npad // 2, 2 * D).copy(),
        iota64=iota64, ident=ident,
        wcb=wcb, w1l=np.asarray(W1l, np.float32), w2lr=w2lr,
        gamma_bc=gamma_bc, beta_bc=beta_bc, b2_bc=b2_bc,
    )
    in_maps = []
    for c in range(N_CORES):
        m = dict(common)
        xc_loc = xp[cp * c:cp * (c + 1)]
        xt = np.empty((D + 1, cp), np.float32)
        xt[0:D] = xc_loc.T
        xt[D] = 1.0
        m["xt"] = xt
        m["gidx"] = plan["gidx_tile"][c]
        m["dstf"] = plan["dstf_tile"][c]
        m["rc"] = plan["rc_tile"][c]
        in_maps.append(m)
    return in_maps


_CACHE = {}


def _get_compiled(edge_index, n_nodes):
    key = (edge_index.tobytes()[:512], edge_index.shape, n_nodes)
    if key not in _CACHE:
        plan = make_plan(edge_index, n_nodes)
        nc = build_program(plan)
        _CACHE[key] = (plan, nc)
    return _CACHE[key]


def run(inputs, trace=False):
    x = np.asarray(inputs["x"], np.float32)
    edge_index = np.asarray(inputs["edge_index"], np.int32)
    n_nodes = x.shape[0]
    plan, nc = _get_compiled(edge_index, n_nodes)
    in_maps = make_inputs(
        plan, x, inputs["W1l"], inputs["W1r"], inputs["b1"], inputs["Wskip"],
        inputs["bskip"], inputs["gamma"], inputs["beta"], inputs["W2l"],
        inputs["W2r"], inputs["b2"], n_nodes)
    res = run_bass_kernel_spmd(
        nc, in_maps, list(range(N_CORES)), trace=trace)
    cp = plan["cp"]
    out = np.empty((n_nodes, OUT), np.float32)
    for c in range(N_CORES):
        lo = cp * c
        hi = min(cp * (c + 1), n_nodes)
        out[lo:hi] = res.results[c]["out"][0:hi - lo]
    return out, res


def kernel(**inputs) -> np.ndarray:
    out, _ = run(inputs)
    return out


# revision 33
# speedup vs baseline: 1.2042x; 1.2042x over previous
"""GeneSAGE (2-layer GraphSAGE + skip + LayerNorm + ELU) on 8 Trainium2 cores.

Edge-parallel by destination range: core c owns nodes [cp*c, cp*(c+1)),
cp=6272. Both convs share ONE edge bucketing: edges grouped window-major
by (64-node dst window, src parity) into 128-slot chunks, chunk
structure common to all 8 cores (SPMD), and ONE shared gather-index
table (pair index src>>1, int16). Per chunk a [128, 64] one-hot
(batched: 16 chunks per DVE instruction via broadcast is_equal against
a resident iota) is matmul'd against the gathered payload. Because a
strip (128-window half) covers two adjacent parity groups with
contiguous chunks, each strip accumulates directly in a PSUM pool tile
(no SBUF agg, no adds); the strip epilogue is a single rc-scale.
Gathers use SWDGE dma_gather round-robin over 4 queues (desc-gen runs
on Q7 core pairs per queue; it is the dominant cost, ~6-10ns/desc).
Conv1 gathers 512B fp32 pair rows [x[2m] | x[2m+1]] and slices the
matmul rhs by parity; conv2 gathers 256B rows of a device-built table
where pair-row m = 16 copies of (p0,p1 of nodes 2m, 2m+1), rhs is a
2-column parity slice (p = h @ W2l is computed per window in the conv1
dense phase, AllGather'd, expanded on DVE, and written back to HBM).
The dense phase (mean transpose/linear/LayerNorm/ELU/p,r) runs per
128-node window interleaved under the conv1 chunk stream with deferred
epilogues.
"""

import numpy as np

import concourse.mybir as mybir
from concourse import bacc, bass, tile
from concourse.bass_utils import run_bass_kernel_spmd

F32 = mybir.dt.float32
I16 = mybir.dt.int16

N_CORES = 8
D = 64
HID = 256
OUT = 2
LN_EPS = 1e-5
B = 16            # chunks per dma_gather
G = 16            # chunks per one-hot DVE instruction
N_SWDGE_Q = 4
DEBUG = False
AF = mybir.ActivationFunctionType
OP = mybir.AluOpType


def make_plan(edge_index: np.ndarray, n_nodes: int):
    cp = int(np.ceil(n_nodes / (N_CORES * 128))) * 128
    nw = cp // 128
    nw64 = cp // 64
    npad = N_CORES * cp
    half = npad // 2
    assert half <= 32768

    src = edge_index[0].astype(np.int64)
    dst = edge_index[1].astype(np.int64)
    E = src.shape[0]

    deg = np.bincount(dst, minlength=npad).astype(np.float64)
    rc = (1.0 / np.maximum(deg, 1.0)).astype(np.float32)
    rc_tile = rc.reshape(N_CORES, nw, 128).transpose(0, 2, 1).copy()

    core = dst // cp
    win64 = (dst % cp) // 64
    parity = (src & 1).astype(np.int64)
    ngrp = 2 * nw64                # window-major groups: (win64, parity)
    g = win64 * 2 + parity
    key = core * ngrp + g
    order = np.argsort(key, kind="stable")
    counts = np.bincount(key, minlength=N_CORES * ngrp).reshape(
        N_CORES, ngrp)
    nch = -(-counts.max(axis=0) // 128)          # [ngrp] common structure
    off = np.zeros(ngrp, np.int64)
    running = 0
    for gg in range(ngrp):
        off[gg] = running
        running += nch[gg]
    c_total = int(running)
    e_slots = c_total * 128

    sk = key[order]
    grp_start = np.searchsorted(sk, np.arange(N_CORES * ngrp))
    rank = np.arange(E) - grp_start[sk]
    g_of = sk % ngrp
    c_of = sk // ngrp
    slot = off[g_of] * 128 + rank

    gidx = np.zeros((N_CORES, e_slots), np.int16)
    dstf = np.full((N_CORES, e_slots), -1.0, np.float32)
    gidx[c_of, slot] = (src[order] >> 1).astype(np.int16)
    dstf[c_of, slot] = (dst[order] % 64).astype(np.float32)

    a = gidx.reshape(N_CORES, e_slots // 16, 16).transpose(0, 2, 1)
    gidx_tile = np.tile(a, (1, 8, 1)).copy()      # [c, 128, J]
    dstf_tile = dstf.reshape(N_CORES, c_total, 128).transpose(0, 2, 1).copy()

    # per-chunk info: (w, s2, par, strip_start, strip_stop). A strip
    # (w, s2) covers both parity groups of win64=2w+s2, which are
    # contiguous in chunk order, so PSUM accumulates per strip directly.
    chunk_info = [None] * c_total
    last_chunk = np.full(nw, -1, np.int64)
    for w in range(nw):
        for s2 in range(2):
            w64 = 2 * w + s2
            present = [w64 * 2 + p for p in range(2) if nch[w64 * 2 + p] > 0]
            if not present:
                continue
            first = int(off[present[0]])
            last = int(off[present[-1]] + nch[present[-1]] - 1)
            for gg in present:
                par = gg % 2
                for c in range(int(off[gg]), int(off[gg] + nch[gg])):
                    chunk_info[c] = (w, s2, par, c == first, c == last)
            last_chunk[w] = max(last_chunk[w], last)
    fire = {}
    for w in range(nw):
        if last_chunk[w] >= 0:
            fire.setdefault(int(last_chunk[w]), []).append(w)

    return dict(cp=cp, nw=nw, nw64=nw64, npad=npad, half=half,
                c_total=c_total, gidx_tile=gidx_tile, dstf_tile=dstf_tile,
                rc_tile=rc_tile, chunk_info=chunk_info, fire=fire)


def build_program(plan):
    cp, nw, npad, half = plan["cp"], plan["nw"], plan["npad"], plan["half"]
    c_total = plan["c_total"]
    J = c_total * 8
    npair = npad // 2
    nblk = npair // 128         # 196 pair-rows per partition
    chunk_info = plan["chunk_info"]
    fire = plan["fire"]

    nc = bacc.Bacc("TRN2", target_bir_lowering=False, debug=False,
                   num_devices=N_CORES, num_swdge_queues=N_SWDGE_Q)

    def inp(name, shape, dt=F32):
        return nc.dram_tensor(name, shape, dt, kind="ExternalInput").ap()

    x2_d = inp("x2", [npair, 2 * D])
    xt_d = inp("xt", [D + 1, cp])
    gidx_d = inp("gidx", [128, J], I16)
    dstf_d = inp("dstf", [128, c_total])
    iota_d = inp("iota64", [128, G, 64])
    ident_d = inp("ident", [128, 128])
    wcb_d = inp("wcb", [D + 1, HID])
    w1l_d = inp("w1l", [D, HID])
    w2lr_d = inp("w2lr", [128, 2 * 2 * OUT])
    gamma_d = inp("gamma_bc", [128, HID])
    beta_d = inp("beta_bc", [128, HID])
    b2_d = inp("b2_bc", [128, OUT])
    rc_d = inp("rc", [128, nw])
    out_d = nc.dram_tensor("out", [cp, OUT], F32, kind="ExternalOutput").ap()
    if DEBUG:
        dagg_d = nc.dram_tensor("dagg", [cp, D], F32,
                                kind="ExternalOutput").ap()
        dpr_d = nc.dram_tensor("dpr", [cp, 2 * OUT], F32,
                               kind="ExternalOutput").ap()
        dpa_d = nc.dram_tensor("dpa", [128, nblk * 2 * OUT], F32,
                               kind="ExternalOutput").ap()

    with tile.TileContext(nc) as tc:
        with (
            tc.tile_pool(name="res", bufs=1) as res,
            tc.tile_pool(name="dram", bufs=1, space="DRAM") as dram,
        ):
            def load(name, shape, src, dt=F32):
                t = res.tile(shape, dt, tag=name)
                nc.sync.dma_start(out=t[:], in_=src[:])
                return t

            gidx_sb = load("gidx", [128, J], gidx_d, I16)
            dstf_sb = load("dstf", [128, c_total], dstf_d)
            iota_sb = load("iota", [128, G, 64], iota_d)
            ident_sb = load("ident", [128, 128], ident_d)
            xt_sb = load("xt", [D + 1, cp], xt_d)
            wcb_sb = load("wcb", [D + 1, HID], wcb_d)
            w1l_sb = load("w1l", [D, HID], w1l_d)
            w2lr_sb = load("w2lr", [128, 2 * 2 * OUT], w2lr_d)
            gamma_sb = load("gamma", [128, HID], gamma_d)
            beta_sb = load("beta", [128, HID], beta_d)
            b2_sb = load("b2", [128, OUT], b2_d)
            rc_sb = load("rc", [128, nw], rc_d)

            pr_sb = res.tile([128, nw, 2 * OUT], F32, tag="prs")
            out_sb = res.tile([128, nw, OUT], F32, tag="outs")
            pa_sb = res.tile([128, nblk, 2 * OUT], F32, tag="pas")

            pk_local = dram.tile([cp, OUT], F32)
            pk_all = dram.tile([128, nblk, 2 * OUT], F32)
            tab2 = dram.tile([128, nblk, D], F32)
            tab2_flat = tab2[:].rearrange("q s d -> (q s) d")

            qi = [0]

            def emit_gathers(table, rowd, gpool, tag):
                gmap = {}
                for b0 in range(0, c_total, B):
                    b1 = min(b0 + B, c_total)
                    g_t = gpool.tile([128, B, rowd], F32, tag=tag)
                    n_idx = (b1 - b0) * 128
                    nc.gpsimd.dma_gather(
                        out_ap=g_t[:, 0:b1 - b0, :],
                        in_ap=table,
                        idxs_ap=gidx_sb[:, b0 * 8:b1 * 8],
                        num_idxs=n_idx,
                        num_idxs_reg=n_idx,
                        elem_size=rowd,
                        single_packet=False,
                        queue_num=qi[0] % N_SWDGE_Q,
                    )
                    qi[0] += 1
                    for c in range(b0, b1):
                        gmap[c] = (g_t, c - b0)
                return gmap

            def onehot_for(c, opool, obufs):
                cb = (c // G) * G
                if cb not in obufs:
                    o_t = opool.tile([128, G, 64], F32, tag="o")
                    n = min(G, c_total - cb)
                    src3 = dstf_sb[:, cb:cb + n].unsqueeze(2).broadcast_to(
                        [128, n, 64])
                    nc.vector.tensor_tensor(
                        out=o_t[:, 0:n, :], in0=iota_sb[:, 0:n, :],
                        in1=src3, op=OP.is_equal)
                    obufs.clear()
                    obufs[cb] = o_t
                return obufs[cb]

            # =================== conv1 ===================
            with (
                tc.tile_pool(name="gpool", bufs=10) as gpool,
                tc.tile_pool(name="opool", bufs=6) as opool,
                tc.tile_pool(name="pwp", bufs=4, space="PSUM") as pwp,
                tc.tile_pool(name="dps1", bufs=1, space="PSUM") as dps1,
                tc.tile_pool(name="dps2", bufs=1, space="PSUM") as dps2,
                tc.tile_pool(name="dwork", bufs=2) as dwork,
                tc.tile_pool(name="mwork", bufs=6) as mwork,
                tc.tile_pool(name="dsmall", bufs=4) as dsmall,
            ):
                gmap = emit_gathers(x2_d, 2 * D, gpool, "g1")
                cur_mean = {}

                def dense(w):
                    mean_sb = cur_mean.pop(w)
                    tp = dps1.tile([64, 128], F32, tag="tp")
                    nc.tensor.transpose(tp[:], mean_sb[:], ident_sb[:])
                    meant = mwork.tile([64, 128], F32, tag="meant")
                    nc.scalar.activation(meant[:], tp[:], AF.Copy)

                    x1p = dps2.tile([128, HID], F32, tag="x1")
                    nc.tensor.matmul(
                        x1p[:], xt_sb[:, 128 * w:128 * (w + 1)], wcb_sb[:],
                        start=True, stop=False)
                    nc.tensor.matmul(x1p[:], meant[:], w1l_sb[:],
                                     start=False, stop=True)

                    # LayerNorm
                    musum = dsmall.tile([128, 1], F32, tag="mu")
                    nc.vector.reduce_sum(out=musum[:], in_=x1p[:],
                                         axis=mybir.AxisListType.X)
                    mu = dsmall.tile([128, 1], F32, tag="muv")
                    nc.vector.tensor_scalar(
                        out=mu[:], in0=musum[:], scalar1=1.0 / HID,
                        scalar2=None, op0=OP.mult)
                    xc = dwork.tile([128, HID], F32, tag="xc")
                    nc.vector.tensor_scalar(
                        out=xc[:], in0=x1p[:], scalar1=mu[:], scalar2=None,
                        op0=OP.subtract)
                    sq = dwork.tile([128, HID], F32, tag="sq")
                    var = dsmall.tile([128, 1], F32, tag="var")
                    nc.vector.scalar_tensor_tensor(
                        out=sq[:], in0=xc[:], scalar=1.0, in1=xc[:],
                        op0=OP.mult, op1=OP.mult, accum_out=var[:])
                    rstd = dsmall.tile([128, 1], F32, tag="rstd")
                    nc.vector.tensor_scalar(
                        out=rstd[:], in0=var[:], scalar1=1.0 / HID,
                        scalar2=LN_EPS, op0=OP.mult, op1=OP.add)
                    nc.scalar.activation(rstd[:], rstd[:], AF.Sqrt)
                    nc.vector.reciprocal(rstd[:], rstd[:])
                    y = dwork.tile([128, HID], F32, tag="y")
                    nc.vector.scalar_tensor_tensor(
                        out=y[:], in0=xc[:], scalar=rstd[:], in1=gamma_sb[:],
                        op0=OP.mult, op1=OP.mult)
                    nc.vector.tensor_tensor(
                        out=y[:], in0=y[:], in1=beta_sb[:], op=OP.add)
                    # ELU = (max(y,0)-1) + exp(min(y,0))
                    m0 = dwork.tile([128, HID], F32, tag="m0")
                    nc.vector.tensor_scalar(
                        out=m0[:], in0=y[:], scalar1=0.0, scalar2=None,
                        op0=OP.min)
                    ex = dwork.tile([128, HID], F32, tag="ex")
                    nc.scalar.activation(ex[:], m0[:], AF.Exp)
                    rm1 = dwork.tile([128, HID], F32, tag="rm1")
                    nc.vector.tensor_scalar(
                        out=rm1[:], in0=y[:], scalar1=0.0, scalar2=-1.0,
                        op0=OP.max, op1=OP.add)
                    h = dwork.tile([128, HID], F32, tag="h")
                    nc.vector.tensor_tensor(
                        out=h[:], in0=rm1[:], in1=ex[:], op=OP.add)

                    # p | r = h @ [W2l | W2r]
                    prp = dps2.tile([128, 2 * OUT], F32, tag="pr")
                    for hh in range(2):
                        tph = dps1.tile([128, 128], F32, tag="tph")
                        nc.tensor.transpose(
                            tph[:], h[:, 128 * hh:128 * (hh + 1)],
                            ident_sb[:])
                        hts = mwork.tile([128, 128], F32, tag="hts")
                        nc.scalar.activation(hts[:], tph[:], AF.Copy)
                        nc.tensor.matmul(
                            prp[:], hts[:], w2lr_sb[:, 4 * hh:4 * (hh + 1)],
                            start=(hh == 0), stop=(hh == 1))
                    nc.scalar.activation(pr_sb[:, w, :], prp[:], AF.Copy)
                    nc.sync.dma_start(
                        out=pk_local[128 * w:128 * (w + 1), :],
                        in_=pr_sb[:, w, 0:OUT])
                    if DEBUG:
                        nc.sync.dma_start(
                            out=dagg_d[128 * w:128 * (w + 1), :],
                            in_=mean_sb[:])
                        nc.sync.dma_start(
                            out=dpr_d[128 * w:128 * (w + 1), :],
                            in_=pr_sb[:, w, :])

                obufs = {}
                pend_dense = []
                pw = None
                for c in range(c_total):
                    info = chunk_info[c]
                    if info is None:
                        continue
                    w, s2, par, s_start, s_stop = info
                    o_t = onehot_for(c, opool, obufs)
                    g_t, col = gmap[c]
                    if s_start:
                        pw = pwp.tile([128, D], F32, tag="pw")
                    nc.tensor.matmul(
                        pw[64 * s2:64 * s2 + 64, :],
                        o_t[:, c - (c // G) * G, :],
                        g_t[:, col, 64 * par:64 * par + 64],
                        start=s_start, stop=s_stop)
                    if s_stop:
                        if w not in cur_mean:
                            mtile = mwork.tile([128, D], F32, tag="mean")
                            cur_mean[w] = mtile
                        nc.vector.tensor_scalar(
                            out=cur_mean[w][64 * s2:64 * s2 + 64, :],
                            in0=pw[64 * s2:64 * s2 + 64, :],
                            scalar1=rc_sb[64 * s2:64 * s2 + 64, w:w + 1],
                            scalar2=None, op0=OP.mult)
                    if c in fire:
                        for wf in fire[c]:
                            pend_dense.append(wf)
                            if len(pend_dense) == 3:
                                dense(pend_dense.pop(0))
                for wf in pend_dense:
                    dense(wf)

            # =============== AllGather p + build conv2 table ===============
            nc.gpsimd.collective_compute(
                "AllGather",
                OP.bypass,
                replica_groups=[list(range(N_CORES))],
                ins=[pk_local.opt()],
                outs=[pk_all.opt()],
            )
            nc.sync.dma_start(out=pa_sb[:], in_=pk_all[:])
            if DEBUG:
                nc.sync.dma_start(out=dpa_d[:], in_=pa_sb[:])
            with tc.tile_pool(name="tabp", bufs=2) as tabp:
                npiece = 4
                pb = nblk // npiece
                for pc in range(npiece):
                    tp_ = tabp.tile([128, pb, 16, 2 * OUT], F32, tag="tp")
                    srcv = pa_sb[:, pb * pc:pb * (pc + 1), :].unsqueeze(
                        2).broadcast_to([128, pb, 16, 2 * OUT])
                    nc.vector.tensor_copy(out=tp_[:], in_=srcv)
                    nc.sync.dma_start(
                        out=tab2[:, pb * pc:pb * (pc + 1), :], in_=tp_[:])

            # =================== conv2 ===================
            with (
                tc.tile_pool(name="gpool2", bufs=10) as gpool2,
                tc.tile_pool(name="opool2", bufs=6) as opool2,
                tc.tile_pool(name="pw2p", bufs=4, space="PSUM") as pw2p,
                tc.tile_pool(name="ewk", bufs=6) as ewk,
            ):
                gmap2 = emit_gathers(tab2_flat, D, gpool2, "g2")
                cur_t = {}

                def epi2(w):
                    t = cur_t.pop(w)
                    nc.vector.tensor_tensor(
                        out=t[:], in0=t[:], in1=pr_sb[:, w, OUT:2 * OUT],
                        op=OP.add)
                    nc.vector.tensor_tensor(
                        out=out_sb[:, w, :], in0=t[:], in1=b2_sb[:],
                        op=OP.add)
                    nc.sync.dma_start(
                        out=out_d[128 * w:128 * (w + 1), :],
                        in_=out_sb[:, w, :])

                obufs2 = {}
                pend2 = []
                pw2 = None
                for c in range(c_total):
                    info = chunk_info[c]
                    if info is None:
                        continue
                    w, s2, par, s_start, s_stop = info
                    o_t = onehot_for(c, opool2, obufs2)
                    g_t, col = gmap2[c]
                    if s_start:
                        pw2 = pw2p.tile([128, OUT], F32, tag="pw2")
                    nc.tensor.matmul(
                        pw2[64 * s2:64 * s2 + 64, :],
                        o_t[:, c - (c // G) * G, :],
                        g_t[:, col, 2 * par:2 * par + 2],
                        start=s_start, stop=s_stop)
                    if s_stop:
                        if w not in cur_t:
                            ttile = ewk.tile([128, OUT], F32, tag="t")
                            cur_t[w] = ttile
                        nc.vector.tensor_scalar(
                            out=cur_t[w][64 * s2:64 * s2 + 64, :],
                            in0=pw2[64 * s2:64 * s2 + 64, :],
                            scalar1=rc_sb[64 * s2:64 * s2 + 64, w:w + 1],
                            scalar2=None, op0=OP.mult)
                    if c in fire:
                        for wf in fire[c]:
                            pend2.append(wf)
                            if len(pend2) == 3:
                                epi2(pend2.pop(0))
                for wf in pend2:
                    epi2(wf)

    nc.compile()
    return nc


def make_inputs(plan, x, W1l, W1r, b1, Wskip, bskip, gamma, beta, W2l, W2r,
                b2, n_nodes):
    cp, half, npad, nw = plan["cp"], plan["half"], plan["npad"], plan["nw"]
    xp = np.zeros((npad, D), np.float32)
    xp[:n_nodes] = np.asarray(x, np.float32)
    wc = np.asarray(W1r, np.float32) + np.asarray(Wskip, np.float32)
    bc = np.asarray(b1, np.float32) + np.asarray(bskip, np.float32)
    wcb = np.concatenate([wc, bc[None, :]], axis=0)
    w2lr_full = np.concatenate(
        [np.asarray(W2l, np.float32), np.asarray(W2r, np.float32)], axis=1)
    w2lr = (
        w2lr_full.reshape(2, 128, 2 * OUT).transpose(1, 0, 2)
        .reshape(128, 2 * 2 * OUT).copy()
    )
    iota64 = np.broadcast_to(
        np.arange(64, dtype=np.float32)[None, None, :],
        (128, G, 64)).copy()
    ident = np.eye(128, dtype=np.float32)
    gamma_bc = np.tile(np.asarray(gamma, np.float32)[None, :], (128, 1))
    beta_bc = np.tile(np.asarray(beta, np.float32)[None, :], (128, 1))
    b2_bc = np.tile(np.asarray(b2, np.float32)[None, :], (128, 1))

    common = dict(
        x2=xp.reshape(npad // 2, 2 * D).copy(),
        iota64=iota64, ident=ident,
        wcb=wcb, w1l=np.asarray(W1l, np.float32), w2lr=w2lr,
        gamma_bc=gamma_bc, beta_bc=beta_bc, b2_bc=b2_bc,
    )
    in_maps = []
    for c in range(N_CORES):
        m = dict(common)
        xc_loc = xp[cp * c:cp * (c + 1)]
        xt = np.empty((D + 1, cp), np.float32)
        xt[0:D] = xc_loc.T
        xt[D] = 1.0
        m["xt"] = xt
        m["gidx"] = plan["gidx_tile"][c]
        m["dstf"] = plan["dstf_tile"][c]
        m["rc"] = plan["rc_tile"][c]
        in_maps.append(m)
    return in_maps


_CACHE = {}


def _get_compiled(edge_index, n_nodes):
    key = (edge_index.tobytes()[:512], edge_index.shape, n_nodes)
    if key not in _CACHE:
        plan = make_plan(edge_index, n_nodes)
        nc = build_program(plan)
        _CACHE[key] = (plan, nc)
    return _CACHE[key]


def run(inputs, trace=False):
    x = np.asarray(inputs["x"], np.float32)
    edge_index = np.asarray(inputs["edge_index"], np.int32)
    n_nodes = x.shape[0]
    plan, nc = _get_compiled(edge_index, n_nodes)
    in_maps = make_inputs(
        plan, x, inputs["W1l"], inputs["W1r"], inputs["b1"], inputs["Wskip"],
        inputs["bskip"], inputs["gamma"], inputs["beta"], inputs["W2l"],
        inputs["W2r"], inputs["b2"], n_nodes)
    res = run_bass_kernel_spmd(
        nc, in_maps, list(range(N_CORES)), trace=trace)
    cp = plan["cp"]
    out = np.empty((n_nodes, OUT), np.float32)
    for c in range(N_CORES):
        lo = cp * c
        hi = min(cp * (c + 1), n_nodes)
        out[lo:hi] = res.results[c]["out"][0:hi - lo]
    return out, res


def kernel(**inputs) -> np.ndarray:
    out, _ = run(inputs)
    return out
